# revision 1
# baseline (speedup 1.0000x reference)
import sys
import numpy as np

sys.path.insert(0, "/opt/trn_rl_repo")

import concourse.bass as bass  # noqa: E402
import concourse.bacc as bacc  # noqa: E402
import concourse.tile as tile  # noqa: E402
from concourse import mybir  # noqa: E402
from concourse.bass_utils import run_bass_kernel_spmd  # noqa: E402

# Problem dims (hardcoded per spec)
N, T, V, C_IN, C_OUT, K, KT = 256, 2048, 9, 16, 3, 5, 9
F_IN = V * C_IN    # 144
F_OUT = V * C_OUT  # 27
N_CORES = 8
N_PER_CORE = N // N_CORES  # 32

F32 = mybir.dt.float32
F16 = mybir.dt.float16

_PROGRAM_CACHE = {}


def _build_program():
    nc = bacc.Bacc()

    # poseT: feature-major input, rows 0..127 = x.T rows 0..127 (per sample)
    poseT = nc.declare_dram_parameter("poseT", [N_PER_CORE, 128, T], F16, isOutput=False)
    # pbs4: per 4-sample group, rows 32k+0..16 = feats 128..143 + ones, rest 0
    pbs4 = nc.declare_dram_parameter("pbs4", [N_PER_CORE // 4, 128, T], F16, isOutput=False)
    waT = nc.declare_dram_parameter("waT", [128, 32], F16, isOutput=False)
    wb4 = nc.declare_dram_parameter("wb4", [128, 128], F16, isOutput=False)
    ct = nc.declare_dram_parameter("ct", [KT, 128, 128], F16, isOutput=False)
    bias4 = nc.declare_dram_parameter("bias4", [128, 1], F32, isOutput=False)
    # raw tile dump: [group, chunk, (4 samples x 32ch), 512t]; host unpacks
    out = nc.declare_dram_parameter(
        "out", [N_PER_CORE // 4, T // 512, 128, 512], F16, isOutput=True)

    NG = N_PER_CORE // 4  # groups of 4 samples
    NC_T = T // 512       # 512-col chunks per sample

    with tile.TileContext(nc) as tc:
        with (
            tc.tile_pool(name="const", bufs=1) as cpool,
            tc.tile_pool(name="poseT", bufs=3) as ppool,
            tc.tile_pool(name="zbuf", bufs=3) as zpool,
            tc.tile_pool(name="outsb", bufs=3) as opool,
            tc.tile_pool(name="psZ", bufs=4, space=bass.MemorySpace.PSUM) as psZ_p,
            tc.tile_pool(name="psO", bufs=4, space=bass.MemorySpace.PSUM) as psO_p,
        ):
            # ---- constants ----
            waT_sb = cpool.tile([128, 32], F16, tag="waT")
            wb4_sb = cpool.tile([128, 128], F16, tag="wb4")
            ct_sb = [
                cpool.tile([128, 128], F16, tag=f"ct{i}", name=f"ct_sb{i}")
                for i in range(KT)
            ]
            bias4_sb = cpool.tile([128, 1], F32, tag="bias4")

            nc.scalar.dma_start(waT_sb[:], waT[:])
            nc.scalar.dma_start(wb4_sb[:], wb4[:])
            for i in range(KT):
                nc.scalar.dma_start(ct_sb[i][:], ct[i])
            nc.scalar.dma_start(bias4_sb[:], bias4[:])

            for g in range(NG):
                # ---- load 4 samples (feature-major already) ----
                pta = []
                for kk in range(4):
                    n = 4 * g + kk
                    pa = ppool.tile([128, T], F16, tag=f"pta{kk}", name=f"pa{kk}")
                    nc.sync.dma_start(pa[:], poseT[n, 0:128, :])
                    pta.append(pa)
                pbs = ppool.tile([128, T], F16, tag="pbs", name="pbs")
                nc.sync.dma_start(pbs[:], pbs4[g])

                # ---- GCN: z[128=(4k x 32ch), t] ----
                zb = zpool.tile([128, T + 8], F16, tag="zb")
                nc.vector.memset(zb[:, 0:4], 0.0)
                nc.vector.memset(zb[:, T + 4:T + 8], 0.0)
                for c in range(NC_T):
                    psZ = psZ_p.tile([128, 512], F32, tag="psZ")
                    sl = slice(c * 512, (c + 1) * 512)
                    # A-chunks first (each clears its own col-group) so chunk
                    # start doesn't wait on the stacked-B load; B accumulates
                    # full-width last.
                    for kk in range(4):
                        nc.tensor.matmul(
                            psZ[32 * kk:32 * kk + 32, :], waT_sb[:], pta[kk][:, sl],
                            start=True, stop=False, tile_position=(0, 32 * kk),
                        )
                    nc.tensor.matmul(
                        psZ[:], wb4_sb[:], pbs[:, sl],
                        start=False, stop=True,
                    )
                    nc.vector.tensor_copy(zb[:, 4 + c * 512:4 + (c + 1) * 512], psZ[:])

                # ---- conv + bias + leaky relu + store ----
                for c in range(NC_T):
                    psO = psO_p.tile([128, 512], F32, tag="psO")
                    for it in range(KT):
                        nc.tensor.matmul(
                            psO[:], ct_sb[it][:],
                            zb[:, c * 512 + it:c * 512 + it + 512],
                            start=(it == 0), stop=(it == KT - 1),
                        )
                    osb = opool.tile([128, 512], F16, tag="osb")
                    nc.scalar.activation(
                        osb[:], psO[:], mybir.ActivationFunctionType.Lrelu,
                        bias=bias4_sb[:, 0:1], alpha=0.01,
                    )
                    eng = nc.scalar if c % 2 == 0 else nc.sync
                    eng.dma_start(out[g, c], osb[:])

    nc.finalize()
    return nc


def _host_consts(A, W_gcn, b_gcn, W_tcn, b_tcn):
    A = np.asarray(A, np.float32)
    W_gcn = np.asarray(W_gcn, np.float32)
    b_gcn = np.asarray(b_gcn, np.float32)
    W_tcn = np.asarray(W_tcn, np.float32)
    b_tcn = np.asarray(b_tcn, np.float32)

    # W_eff[(v,c),(w,o)] = sum_k W_gcn[k,o,c] A[k,v,w]
    W_eff = np.einsum("koc,kvw->vcwo", W_gcn, A).reshape(F_IN, F_OUT).astype(np.float32)
    b_eff = np.einsum("ko,kw->wo", b_gcn, A.sum(axis=1)).reshape(F_OUT).astype(np.float32)

    waT = np.zeros((128, 32), np.float32)
    waT[:, :F_OUT] = W_eff[:128]
    wb1 = np.zeros((17, 32), np.float32)
    wb1[:16, :F_OUT] = W_eff[128:]
    wb1[16, :F_OUT] = b_eff  # multiplied by the ones row
    wb4 = np.zeros((128, 128), np.float32)
    for kk in range(4):
        wb4[32 * kk:32 * kk + 17, 32 * kk:32 * kk + 32] = wb1

    # conv taps: C_tau[i,o] = W_tcn[o,i,4-tau]; block-diag over (4 samples x 32) with
    # within-32 block-diag over joints w: (w,i) -> (w,o)
    ct = np.zeros((KT, 128, 128), np.float32)
    for it, tau in enumerate(range(-4, 5)):
        Ct = W_tcn[:, :, 4 - tau, 0].T  # [i, o]
        blk = np.zeros((32, 32), np.float32)
        for w in range(V):
            blk[3 * w:3 * w + 3, 3 * w:3 * w + 3] = Ct
        for kk in range(4):
            ct[it, 32 * kk:32 * kk + 32, 32 * kk:32 * kk + 32] = blk

    bias4 = np.zeros((128, 1), np.float32)
    for kk in range(4):
        bias4[32 * kk:32 * kk + F_OUT, 0] = np.tile(b_tcn, V)

    f16 = np.float16
    return waT.astype(f16), wb4.astype(f16), ct.astype(f16), bias4


def _host_transpose(pose):
    # -> poseT [N, 128, T] (feats 0..127) and pbs4 [N//4, 128, T]
    # (rows 32k+0..15 = feats 128..143 of sample 4g+k, row 32k+16 = ones)
    x16 = np.swapaxes(pose.astype(np.float16), 1, 2)  # [N, 144, T]
    poseT = np.ascontiguousarray(x16[:, :128, :])
    pbs4 = np.zeros((N // 4, 128, T), np.float16)
    for kk in range(4):
        pbs4[:, 32 * kk:32 * kk + 16, :] = x16[kk::4][:, 128:144, :]
        pbs4[:, 32 * kk + 16, :] = np.float16(1.0)
    return poseT, pbs4


def _run(inputs, **spmd_kwargs):
    pose = np.asarray(inputs["pose_feats"], np.float32)
    poseT, pbs4 = _host_transpose(pose)
    waT, wb4, ct, bias4 = _host_consts(
        inputs["A"], inputs["W_gcn"], inputs["b_gcn"], inputs["W_tcn"], inputs["b_tcn"]
    )

    if "prog" not in _PROGRAM_CACHE:
        _PROGRAM_CACHE["prog"] = _build_program()
    nc = _PROGRAM_CACHE["prog"]

    in_maps = []
    for i in range(N_CORES):
        in_maps.append({
            "poseT": poseT[i * N_PER_CORE:(i + 1) * N_PER_CORE],
            "pbs4": pbs4[i * (N_PER_CORE // 4):(i + 1) * (N_PER_CORE // 4)],
            "waT": waT, "wb4": wb4,
            "ct": ct, "bias4": bias4,
        })
    res = run_bass_kernel_spmd(nc, in_maps, list(range(N_CORES)), **spmd_kwargs)
    outs = [res.results[i]["out"] for i in range(N_CORES)]
    full = np.concatenate(outs, axis=0)          # [N//4, T//512, 128, 512]
    full = full.reshape(N // 4, T // 512, 4, 32, 512)[:, :, :, 0:F_OUT, :]
    # -> [N//4, 4, T//512, 512, 27] -> [N, T, 27]; cast before reshape so the
    # transpose materializes once, directly in f32
    full = full.transpose(0, 2, 1, 4, 3).astype(np.float32).reshape(N, T, F_OUT)
    return full, res


def kernel(**inputs) -> np.ndarray:
    out, _ = _run(inputs)
    return out



# revision 2
# speedup vs baseline: 2.0853x; 2.0853x over previous
import sys

import numpy as np
import ml_dtypes

sys.path.insert(0, "/opt/trn_rl_repo")

import concourse.bass as bass  # noqa: E402
import concourse.bacc as bacc  # noqa: E402
import concourse.tile as tile  # noqa: E402
from concourse import mybir  # noqa: E402
from concourse.bass_utils import run_bass_kernel_spmd  # noqa: E402

# Problem dims (hardcoded per spec)
N, T, V, C_IN, C_OUT, K, KT = 256, 2048, 9, 16, 3, 5, 9
F_IN = V * C_IN    # 144
F_OUT = V * C_OUT  # 27
N_CORES = 8
NS = N // N_CORES  # 32 samples per core

WIN = 120          # conv output columns per window
NW = 18            # windows: 17*120 + 8 = 2048
WC = 128           # zT window length (WIN + 8 halo)

F32 = mybir.dt.float32
F16 = mybir.dt.float16
F8 = mybir.dt.float8e3
E3 = ml_dtypes.float8_e3m4

_PROGRAM_CACHE = {}


def _build_program():
    nc = bacc.Bacc()

    # window-major inputs: pw[k, feat(128), 128*s + j] with t = 120k - 4 + j
    # (zero-padded outside [0, T)); x2w packs feats 128..143 of 8 samples per
    # 128-partition group: [k, 16*(s%8) + c, 128*(s//8) + j]
    pw = nc.declare_dram_parameter("pw", [NW, 128, NS * WC], F8, isOutput=False)
    x2w = nc.declare_dram_parameter("x2w", [NW, 128, 4 * WC], F8, isOutput=False)
    weff1 = nc.declare_dram_parameter("weff1", [128, F_OUT], F16, isOutput=False)
    w2big = nc.declare_dram_parameter("w2big", [128, 216], F16, isOutput=False)
    ball = nc.declare_dram_parameter("ball", [128, 9 * WIN], F16, isOutput=False)
    # 3 bias variants: window0 (rows 0-3 zero), interior, window17 (rows 12+ zero)
    beff3 = nc.declare_dram_parameter("beff3", [128, 3 * 432], F32, isOutput=False)
    btcn = nc.declare_dram_parameter("btcn", [128, 3], F32, isOutput=False)
    # raw dump: [window, time-in-window, 27*s + 3*w + o']; host unpacks
    out = nc.declare_dram_parameter("out", [NW, WIN, 27 * NS], F16, isOutput=True)

    with tile.TileContext(nc) as tc:
        with (
            tc.tile_pool(name="const", bufs=1) as cpool,
            tc.tile_pool(name="pw", bufs=3) as pwp,
            tc.tile_pool(name="x2", bufs=3) as x2p,
            tc.tile_pool(name="zt", bufs=2) as ztp,
            tc.tile_pool(name="osb", bufs=3) as osp,
            tc.tile_pool(name="psG", bufs=4, space=bass.MemorySpace.PSUM) as psG,
            tc.tile_pool(name="psC", bufs=4, space=bass.MemorySpace.PSUM) as psC,
        ):
            weff1_sb = cpool.tile([128, F_OUT], F16, tag="weff1")
            w2big_sb = cpool.tile([128, 216], F16, tag="w2big")
            ball_sb = cpool.tile([128, 9 * WIN], F16, tag="ball")
            beff_sb = cpool.tile([128, 3 * 432], F32, tag="beff3")
            btcn_sb = cpool.tile([128, 3], F32, tag="btcn")
            nc.scalar.dma_start(weff1_sb[:], weff1[:])
            nc.scalar.dma_start(w2big_sb[:], w2big[:])
            nc.scalar.dma_start(ball_sb[:], ball[:])
            nc.scalar.dma_start(beff_sb[:], beff3[:])
            nc.scalar.dma_start(btcn_sb[:], btcn[:])

            for k in range(NW):
                pwt = pwp.tile([128, NS * WC], F8, tag="pw")
                nc.sync.dma_start(pwt[:], pw[k])
                x2t = x2p.tile([128, 4 * WC], F8, tag="x2")
                nc.sync.dma_start(x2t[:], x2w[k])

                # GCN: zT[t, 27*s + ch] for t = 120k - 4 + p
                zt = ztp.tile([128, 27 * NS], F16, tag="zt")
                bcol = 0 if k == 0 else (2 * 432 if k == NW - 1 else 432)
                for h in range(2):  # 16 samples per psum bank
                    ps = psG.tile([128, 432], F32, tag="g")
                    for sl in range(16):
                        s = 16 * h + sl
                        nc.tensor.matmul(
                            ps[:, 27 * sl:27 * sl + 27],
                            pwt[:, WC * s:WC * s + WC], weff1_sb[:],
                            start=(sl == 0), stop=False,
                        )
                    for g2 in range(2):
                        g = 2 * h + g2
                        nc.tensor.matmul(
                            ps[:, 216 * g2:216 * g2 + 216],
                            x2t[:, WC * g:WC * g + WC], w2big_sb[:],
                            start=False, stop=(g2 == 1),
                        )
                    nc.vector.tensor_tensor(
                        zt[:, 432 * h:432 * h + 432], ps[:],
                        beff_sb[:, bcol:bcol + 432], mybir.AluOpType.add,
                    )

                # conv: out[120k + i, (w, o')] via banded-Toeplitz stationary
                ot = osp.tile([128, 27 * NS], F16, tag="osb")
                for op_ in range(3):
                    pc = psC.tile([128, 9 * NS], F32, tag="c")
                    for o in range(3):
                        q = 3 * o + op_
                        nc.tensor.matmul(
                            pc[0:WIN, :],
                            ball_sb[:, WIN * q:WIN * q + WIN],
                            zt[:, o:27 * NS:3],
                            start=(o == 0), stop=(o == 2),
                        )
                    nc.scalar.activation(
                        ot[0:WIN, op_:27 * NS:3], pc[0:WIN, :],
                        mybir.ActivationFunctionType.Lrelu,
                        bias=btcn_sb[0:WIN, op_:op_ + 1], alpha=0.01,
                    )
                nc.gpsimd.dma_start(out[k], ot[0:WIN, :])

    nc.finalize()
    return nc


def _host_consts(A, W_gcn, b_gcn, W_tcn, b_tcn):
    A = np.asarray(A, np.float32)
    W_gcn = np.asarray(W_gcn, np.float32)
    b_gcn = np.asarray(b_gcn, np.float32)
    W_tcn = np.asarray(W_tcn, np.float32)
    b_tcn = np.asarray(b_tcn, np.float32)

    # W_eff[(v,c),(w,o)] = sum_k W_gcn[k,o,c] A[k,v,w]
    W_eff = np.einsum("koc,kvw->vcwo", W_gcn, A).reshape(F_IN, F_OUT)
    b_eff = np.einsum("ko,kw->wo", b_gcn, A.sum(axis=1)).reshape(F_OUT)

    weff1 = W_eff[:128].astype(np.float16)                        # [128, 27]
    w2big = np.zeros((128, 216), np.float16)
    for sm in range(8):
        w2big[16 * sm:16 * sm + 16, 27 * sm:27 * sm + 27] = W_eff[128:144]

    # conv taps: out[t,(w,o')] = sum_tau sum_o Ctaps[tau][o,o'] z[t+tau,(w,o)]
    Ctaps = {tau: W_tcn[:, :, 4 - tau, 0].T for tau in range(-4, 5)}
    ball = np.zeros((128, 9 * WIN), np.float16)
    for o in range(3):
        for op_ in range(3):
            q = 3 * o + op_
            for d in range(-4, 5):
                val = np.float16(Ctaps[d][o, op_])
                ii = np.arange(WIN)
                ball[ii + d + 4, WIN * q + ii] = val

    beff_row = np.tile(b_eff, 16).astype(np.float32)              # [432]
    beff3 = np.zeros((128, 3 * 432), np.float32)
    beff3[4:, 0:432] = beff_row          # window 0: t = p - 4 >= 0 only
    beff3[:, 432:864] = beff_row         # interior
    beff3[0:12, 864:1296] = beff_row     # window 17: t < T only

    btcn = np.zeros((128, 3), np.float32)
    btcn[:, :] = b_tcn[None, :]
    return weff1, w2big, ball, beff3, btcn


def _host_windows(pose):
    # pose [N, T, 144] f32 -> per-core window-major fp8 arrays
    x8 = np.ascontiguousarray(pose.transpose(0, 2, 1)).astype(E3)  # [N, 144, T]
    Q = np.zeros((N, F_IN, 2304), E3)
    Q[:, :, 4:4 + T] = x8
    sN, sF, sT = Q.strides
    Wv = np.lib.stride_tricks.as_strided(
        Q, shape=(N, F_IN, NW, WC), strides=(sN, sF, 120 * sT, sT))
    pws, x2s = [], []
    for c in range(N_CORES):
        Wc = Wv[32 * c:32 * c + 32]                       # [32, 144, 18, 128]
        pw = np.ascontiguousarray(
            Wc[:, :128].transpose(2, 1, 0, 3)).reshape(NW, 128, NS * WC)
        w2 = Wc[:, 128:144].reshape(4, 8, 16, NW, WC)     # [g, sm, c, k, j]
        x2 = np.ascontiguousarray(
            w2.transpose(3, 1, 2, 0, 4)).reshape(NW, 128, 4 * WC)
        pws.append(pw)
        x2s.append(x2)
    return pws, x2s


def _run(inputs, **spmd_kwargs):
    pose = np.asarray(inputs["pose_feats"], np.float32)
    pws, x2s = _host_windows(pose)
    weff1, w2big, ball, beff3, btcn = _host_consts(
        inputs["A"], inputs["W_gcn"], inputs["b_gcn"], inputs["W_tcn"], inputs["b_tcn"]
    )

    if "prog" not in _PROGRAM_CACHE:
        _PROGRAM_CACHE["prog"] = _build_program()
    nc = _PROGRAM_CACHE["prog"]

    in_maps = []
    for i in range(N_CORES):
        in_maps.append({
            "pw": pws[i], "x2w": x2s[i],
            "weff1": weff1, "w2big": w2big, "ball": ball,
            "beff3": beff3, "btcn": btcn,
        })
    res = run_bass_kernel_spmd(nc, in_maps, list(range(N_CORES)), **spmd_kwargs)
    outs = [res.results[i]["out"] for i in range(N_CORES)]
    full = np.stack(outs, axis=0)                 # [8, 18, 120, 864]
    full = full.reshape(N_CORES, NW, WIN, NS, F_OUT)
    full = full.transpose(0, 3, 1, 2, 4).reshape(N, NW * WIN, F_OUT)
    return full[:, :T].astype(np.float32), res


def kernel(**inputs) -> np.ndarray:
    out, _ = _run(inputs)
    return out
